# revision 5
# baseline (speedup 1.0000x reference)
"""Single-head causal attention (B=4, T=4096, C=1024, H=64) on 8 trn2 cores.

Sharding: each core owns one (batch b = i//2, query-interleave j = i%2) pair.
Queries of core (b, j) are the 8 interleaved 256-row chunks (2s+j)*256 of
batch b, which balances causal-attention work exactly across the two cores
of a batch.  Every core receives the full (transposed, bf16) x of its batch
and computes K/V for all 4096 rows; Q only for its own 2048 rows.

Device layout choices:
  - x is passed pre-transposed [C, T] so qT/kT/vT come straight out of PE
    matmuls (contraction over C on partitions).
  - scores are computed transposed [k, q] (K=64 contraction over H), softmax
    runs without max-subtraction (randn-scaled scores are bounded ~|5|), the
    denominator comes free via an all-ones 65th column on V-natural.
  - V is moved to natural [k, H] layout with PE transposes.
"""

import sys

sys.path.insert(0, "/opt/trn_rl_repo")

from contextlib import ExitStack

import ml_dtypes
import numpy as np

import concourse.bass as bass
import concourse.mybir as mybir
import concourse.tile as tile_mod
from concourse.bass_utils import run_bass_kernel_spmd
from concourse.tile import TileContext
from concourse.vector_clock import ScopedClock

# ---------------------------------------------------------------------------
# Workaround: this walrus accepts only ONE sync wait per Drain instruction.
# Split the TileContext exit-drain's waits across multiple drains.
# ---------------------------------------------------------------------------


def _patched_drain_and_barrier(self, tick_clock, wait_clock):
    drain_inst = self.nc.sync.drain()
    wait_clock.add_sem_waits(
        drain_inst.ins, ScopedClock({None: tick_clock.global_clock})
    )
    si = drain_inst.ins.sync_info
    waits = list(si.on_wait or []) if si is not None else []
    if len(waits) > 1:
        si.on_wait = waits[:1]
        for w in waits[1:]:
            d = self.nc.sync.drain()
            dsi = d.ins.sync_info
            if dsi is None:
                d.ins.sync_info = mybir.SyncInfo(on_wait=[w], on_update=[])
            else:
                dsi.on_wait = [w]

    self.nc.all_engine_barrier()
    assert self.sems is not None
    popped = self.nc._tile_sem_poison_stack.pop()
    assert popped is self._sem_poison
    self.nc.clear_and_free_semaphores(list(self.sems.allocated().values()))
    self.nc.all_engine_barrier()


tile_mod.TileContext._drain_and_barrier = _patched_drain_and_barrier


def _split_sync_waits(nc):
    """Rewrite any instruction carrying >1 sync wait into a chain of
    single-wait nops (same engine, inserted just before it)."""
    f = nc.m.functions[0]
    created = []  # names of nops we created (they get appended to cur_bb)

    plans = []  # (block, list of (inst_name, extra_waits))
    for blk in f.blocks:
        insts = list(blk.instructions)
        plan = {}
        for inst in insts:
            si = inst.sync_info
            waits = list(si.on_wait or []) if si is not None else []
            if len(waits) > 1:
                plan[inst.name] = waits[:-1]
                si.on_wait = waits[-1:]
        if plan:
            plans.append((blk, plan))

    nop_map = {}  # inst_name -> list of nop instructions
    for blk, plan in plans:
        for iname, extra in plan.items():
            nops = []
            for w in extra:
                eng_inst = None
                # find engine of target instruction
                eng_type = nc.inst_map[iname].engine
                bi = nc.engines[eng_type].nop(nofuse=True)
                bi.ins.sync_info = mybir.SyncInfo(on_wait=[w], on_update=[])
                created.append(bi.ins.name)
                nops.append(bi.ins)
            nop_map[iname] = nops

    created_set = set(created)
    for blk in f.blocks:
        newl = []
        for inst in blk.instructions:
            if inst.name in created_set:
                continue  # remove from wherever the builder appended it
            if inst.name in nop_map:
                newl.extend(nop_map[inst.name])
            newl.append(inst)
        blk.instructions = newl

# ---------------------------------------------------------------------------

B, T, C, H = 4, 4096, 1024, 64
NCORES = 8
TQ = T // 2          # queries per core
NSLOT = 8            # 256-query slots per core
QS = TQ // NSLOT     # 256
CB = C // 128        # 8 contraction chunks
NRT = T // 512       # 8 row tiles for k/v generation
BF16 = mybir.dt.bfloat16
F32 = mybir.dt.float32
EXPF = mybir.ActivationFunctionType.Exp

_prog_cache = {}


def _build_program():
    nc = bass.Bass("TRN2", target_bir_lowering=False, debug=False,
                   num_devices=NCORES)

    xt_d = nc.dram_tensor("xt", [C, T], BF16, kind="ExternalInput")
    xqt_d = nc.dram_tensor("xqt", [C, TQ], BF16, kind="ExternalInput")
    wkv_d = nc.dram_tensor("wkv", [C, 128], BF16, kind="ExternalInput")
    wq_d = nc.dram_tensor("wq", [C, H], BF16, kind="ExternalInput")
    mask_d = nc.dram_tensor("mask", [128, 1024], BF16, kind="ExternalInput")
    id_d = nc.dram_tensor("ident", [64, 64], BF16, kind="ExternalInput")
    y_d = nc.dram_tensor("y", [TQ, H], F32, kind="ExternalOutput")

    with TileContext(nc) as tc, ExitStack() as ctx:
        const_p = ctx.enter_context(tc.tile_pool(name="const", bufs=1))
        xt_p = ctx.enter_context(tc.tile_pool(name="xt", bufs=1))
        big_p = ctx.enter_context(tc.tile_pool(name="big", bufs=1))
        vtmp_p = ctx.enter_context(tc.tile_pool(name="vtmp", bufs=2))
        exp_p = ctx.enter_context(tc.tile_pool(name="exp", bufs=3))
        out_p = ctx.enter_context(tc.tile_pool(name="outs", bufs=4))
        pm_p = ctx.enter_context(tc.tile_pool(name="pmisc", bufs=2, space="PSUM"))
        ps_p = ctx.enter_context(tc.tile_pool(name="pscore", bufs=2, space="PSUM"))
        po_p = ctx.enter_context(tc.tile_pool(name="pout", bufs=2, space="PSUM"))

        # constants
        wkv_sb = const_p.tile([128, CB, 128], BF16, tag="wkv")
        nc.sync.dma_start(out=wkv_sb[:],
                          in_=wkv_d.ap().rearrange("(c p) w -> p c w", p=128))
        wq_sb = const_p.tile([128, CB, H], BF16, tag="wq")
        nc.sync.dma_start(out=wq_sb[:],
                          in_=wq_d.ap().rearrange("(c p) w -> p c w", p=128))
        mask_sb = const_p.tile([128, 1024], BF16, tag="mask")
        nc.sync.dma_start(out=mask_sb[:], in_=mask_d.ap())
        id_sb = const_p.tile([64, 64], BF16, tag="ident")
        nc.sync.dma_start(out=id_sb[:], in_=id_d.ap())

        # big persistent sbuf tensors
        xt_sb = xt_p.tile([128, CB, NRT, 512], BF16, tag="xt")
        xqt_sb = xt_p.tile([128, CB, 4, 512], BF16, tag="xqt")
        kt_sb = big_p.tile([64, T], BF16, tag="kt")
        qt_sb = big_p.tile([64, TQ], BF16, tag="qt")
        vnat_sb = big_p.tile([128, T // 128, H + 1], BF16, tag="vnat")
        nc.gpsimd.memset(vnat_sb[:], 1.0)

        xt_view = xt_d.ap().rearrange("(c p) (r f) -> c p r f", p=128, f=512)
        xqt_view = xqt_d.ap().rearrange("(c p) (r f) -> c p r f", p=128, f=512)

        def load_xt(rt):
            for c in range(CB):
                nc.sync.dma_start(out=xt_sb[:, c, rt, :], in_=xt_view[c, :, rt, :])

        def load_xqt(qt):
            for c in range(CB):
                nc.sync.dma_start(out=xqt_sb[:, c, qt, :], in_=xqt_view[c, :, qt, :])

        def kv_gen(rt):
            pkv = pm_p.tile([128, 512], F32, tag="pm")
            for c in range(CB):
                nc.tensor.matmul(pkv[:], lhsT=wkv_sb[:, c, :],
                                 rhs=xt_sb[:, c, rt, :],
                                 start=(c == 0), stop=(c == CB - 1))
            nc.vector.tensor_copy(kt_sb[:, rt * 512:(rt + 1) * 512], pkv[0:64, :])
            vt = vtmp_p.tile([64, 512], BF16, tag="vt")
            nc.vector.tensor_copy(vt[:], pkv[64:128, :])
            for t in range(4):
                kb = rt * 4 + t
                pt = pm_p.tile([128, 64], BF16, tag="pm")
                nc.tensor.transpose(pt[:], vt[:, t * 128:(t + 1) * 128], id_sb[:])
                nc.vector.tensor_copy(vnat_sb[:, kb, 0:H], pt[:])

        def q_gen(qt):
            pq = pm_p.tile([64, 512], F32, tag="pm")
            for c in range(CB):
                nc.tensor.matmul(pq[:], lhsT=wq_sb[:, c, :],
                                 rhs=xqt_sb[:, c, qt, :],
                                 start=(c == 0), stop=(c == CB - 1))
            nc.vector.tensor_copy(qt_sb[:, qt * 512:(qt + 1) * 512], pq[:])

        def attention(s):
            rhs_q = qt_sb[:, s * QS:(s + 1) * QS]
            po = [po_p.tile([128, H + 1], F32, tag="po", name=f"po{s}_{h}")
                  for h in range(2)]
            for g in range(s + 1):
                ps = ps_p.tile([128, 1024], F32, tag="ps")
                for f in range(4):
                    kb = 4 * g + f
                    nc.tensor.matmul(ps[:, f * 256:(f + 1) * 256],
                                     lhsT=kt_sb[:, kb * 128:(kb + 1) * 128],
                                     rhs=rhs_q, start=True, stop=True)
                ex = exp_p.tile([128, 1024], BF16, tag="ex")
                nc.scalar.activation(ex[:], ps[:], EXPF)
                if g == s:
                    nc.vector.tensor_mul(ex[:], ex[:], mask_sb[:])
                for f in range(4):
                    kb = 4 * g + f
                    for h in range(2):
                        nc.tensor.matmul(
                            po[h][:],
                            lhsT=ex[:, f * 256 + h * 128: f * 256 + (h + 1) * 128],
                            rhs=vnat_sb[:, kb, :],
                            start=(g == 0 and f == 0),
                            stop=(g == s and f == 3),
                            skip_group_check=True)
            for h in range(2):
                rcp = out_p.tile([128, 1], F32, tag="rcp")
                nc.vector.reciprocal(rcp[:], po[h][:, H:H + 1])
                osb = out_p.tile([128, H], F32, tag="osb")
                nc.vector.tensor_scalar_mul(osb[:], po[h][:, 0:H], rcp[:])
                nc.sync.dma_start(
                    out=y_d[s * QS + h * 128: s * QS + (h + 1) * 128, :],
                    in_=osb[:])

        for s in range(NSLOT):
            load_xt(s)
            if s % 2 == 0:
                load_xqt(s // 2)
            kv_gen(s)
            if s % 2 == 0:
                q_gen(s // 2)
            attention(s)

    _split_sync_waits(nc)
    return nc


def _host_inputs(x, Wq, Wk, Wv):
    """Build the 8 per-core input maps from full fp32 inputs."""
    bf = ml_dtypes.bfloat16
    scale = H ** -0.5
    wkv = np.concatenate([Wk, Wv], axis=1).astype(bf)
    wq = (Wq * scale).astype(bf)
    ident = np.eye(64, dtype=bf)

    p = np.arange(128)[:, None, None]
    f = np.arange(4)[None, :, None]
    ff = np.arange(256)[None, None, :]
    masks = []
    for j in range(2):
        m = (p <= ff + 128 * (2 * j - f)).astype(bf)
        masks.append(np.ascontiguousarray(m.reshape(128, 1024)))

    in_maps = []
    for i in range(NCORES):
        b, j = i // 2, i % 2
        xt = np.ascontiguousarray(x[b].T).astype(bf)
        cols = np.concatenate(
            [np.arange((2 * s + j) * QS, (2 * s + j + 1) * QS)
             for s in range(NSLOT)])
        xqt = np.ascontiguousarray(xt[:, cols])
        in_maps.append({
            "xt": xt, "xqt": xqt, "wkv": wkv, "wq": wq,
            "mask": masks[j], "ident": ident,
        })
    return in_maps


def _gather(results):
    out = np.empty((B, T, H), np.float32)
    for i in range(NCORES):
        b, j = i // 2, i % 2
        y = results[i]["y"]
        for s in range(NSLOT):
            g = (2 * s + j) * QS
            out[b, g:g + QS, :] = y[s * QS:(s + 1) * QS, :]
    return out


def _run_sharded(x, Wq, Wk, Wv, trace=False, **kw):
    if "prog" not in _prog_cache:
        _prog_cache["prog"] = _build_program()
    nc = _prog_cache["prog"]
    in_maps = _host_inputs(x, Wq, Wk, Wv)
    res = run_bass_kernel_spmd(nc, in_maps, list(range(NCORES)),
                               trace=trace, **kw)
    return _gather(res.results), res


def kernel(x, Wq, Wk, Wv):
    out, _ = _run_sharded(x, Wq, Wk, Wv, trace=False)
    return out
